# revision 12
# baseline (speedup 1.0000x reference)
"""Trainium2 Bass kernel for nn_KVCacheHybrid (quantized KV-cache scatter-update).

Reference semantics (per cache, k and v independently):
  1. 4-bit affine quantize along L (scales/zeros reduce over B,H,D per l)
  2. dequantize, scatter new rows at input_pos, re-quantize, dequantize.

Key observations that shape this kernel:
  * After the first quantize/dequant round-trip, codes 0 and 15 are attained in
    every l-slice, so the second-pass min/max for non-updated l are exactly the
    dequant grid endpoints; the second-pass scale/zero differ from the first by
    <= ~2 ulp (a ~1e-5 absolute output shift).  At fp16 output precision that
    is invisible, so this kernel reuses (s1, mn1) directly: out = q*s1 + mn1.
  * For non-updated l the second-pass codes equal the first-pass codes, so the
    device only computes q = rne((x - mn1) / s1) and the affine above.
  * Rows at input_pos depend only on k_val/v_val (0.5 MB) — computed exactly on
    the host and spliced into the gathered output.

Sharding: L axis across 8 cores (512 l's each).  The per-l reduction is then
fully core-local — no collectives.

Performance structure (vs the 244us h-major/f32 variant):
  * Inputs are uploaded l-major ([B, LC, H, D]) so each DMA partition line is
    H*D*4 = 16 KiB contiguous; the h-major layout produced 512 B packets which
    saturated the 16 DMA engines on per-packet overhead (~30ns/packet).
  * Output is written as fp16, halving write traffic; values are exact
    q*s1+mn1 rounded to fp16 (rel err ~2^-11, far inside the 2e-2 gate).
    The host upcasts while re-transposing.
  * f32->int8 output conversion on ACT/DVE rounds to nearest-even (verified on
    HW, including ties), so quantize is ONE fused op: q_i8 = Act(x*inv1 + nb1);
    no separate magic-constant rounding pass.
  * Work is issued per 2 MB b-half ([128 l, 4096]): DVE runs min/max reduces
    (4.34us each, 139us total — the compute wall), ACT runs quantize (f32->i8)
    and dequant (i8->f16); the last chunks' dequants run on DVE instead, which
    is idle once reduces drain, trimming the ACT tail.
"""

import numpy as np
from contextlib import ExitStack

import concourse.bass as bass
import concourse.bacc as bacc
import concourse.tile as tile
from concourse import mybir
from concourse.bass_utils import run_bass_kernel_spmd

F32 = mybir.dt.float32
F16 = mybir.dt.float16
I8 = mybir.dt.int8
ALU = mybir.AluOpType
AXIS = mybir.AxisListType
ACTF = mybir.ActivationFunctionType

B, H, L, D = 2, 32, 4096, 128
N_CORES = 8
LC = L // N_CORES          # 512 l's per core
LCHUNK = 128               # l's per partition-tile
HALF = H * D               # 4096 elements per half-line (one b)
C15 = float(np.float32(1.0 / 15.0))

_BUILD_CACHE = {}


def _build(lc=LC):
    """Builds the per-core SPMD program; identical on all cores."""
    nc = bacc.Bacc("TRN2", target_bir_lowering=False, debug=False,
                   num_devices=N_CORES)
    k = nc.dram_tensor("k", [B, lc, H, D], F32, kind="ExternalInput").ap()
    v = nc.dram_tensor("v", [B, lc, H, D], F32, kind="ExternalInput").ap()
    out = nc.dram_tensor("out", [2, B, lc, H, D], F16,
                         kind="ExternalOutput").ap()

    n_chunks = lc // LCHUNK
    n_groups = 2 * n_chunks

    with tile.TileContext(nc) as tc, ExitStack() as ctx:
        xpool = ctx.enter_context(tc.tile_pool(name="x", bufs=4))
        qpool = ctx.enter_context(tc.tile_pool(name="q", bufs=4))
        opool = ctx.enter_context(tc.tile_pool(name="o", bufs=4))
        cpool = ctx.enter_context(tc.tile_pool(name="c", bufs=3))

        group = 0
        for ci, src in enumerate((k, v)):
            for lchunk in range(n_chunks):
                l0 = lchunk * LCHUNK
                # tail groups: DVE is idle once reduces drain; move dequant
                # there to keep ACT off the critical path at the end.
                tail = group >= n_groups - 2

                # ---- load one [128, (b h d)] tile via 1 MB sub-DMAs -----
                x2 = xpool.tile([128, B * HALF], F32, tag="x")
                x4 = x2[:].rearrange("l (b h d) -> l b h d", b=B, h=H)
                for b in range(B):
                    for s in range(2):
                        h0, h1 = s * (H // 2), (s + 1) * (H // 2)
                        nc.sync.dma_start(
                            out=x4[:, b, h0:h1, :],
                            in_=src[b, l0:l0 + LCHUNK, h0:h1, :])

                # ---- per-l min/max --------------------------------------
                mx1 = cpool.tile([128, 1], F32, tag="mx1")
                mn1 = cpool.tile([128, 1], F32, tag="mn1")
                if group == 0:
                    # split the very first reduce so DVE starts as soon as
                    # the first 1 MB lands instead of waiting for 4 MB.
                    pm = cpool.tile([128, 4], F32, tag="pm")
                    nc.vector.tensor_reduce(pm[:, 0:1], x2[:, 0:HALF // 2],
                                            axis=AXIS.X, op=ALU.max)
                    nc.vector.tensor_reduce(pm[:, 1:2], x2[:, 0:HALF // 2],
                                            axis=AXIS.X, op=ALU.min)
                    nc.vector.tensor_reduce(pm[:, 2:3], x2[:, HALF // 2:],
                                            axis=AXIS.X, op=ALU.max)
                    nc.vector.tensor_reduce(pm[:, 3:4], x2[:, HALF // 2:],
                                            axis=AXIS.X, op=ALU.min)
                    nc.vector.tensor_tensor(mx1[:], pm[:, 0:1], pm[:, 2:3],
                                            op=ALU.max)
                    nc.vector.tensor_tensor(mn1[:], pm[:, 1:2], pm[:, 3:4],
                                            op=ALU.min)
                else:
                    nc.vector.tensor_reduce(mx1[:], x2[:], axis=AXIS.X,
                                            op=ALU.max)
                    nc.vector.tensor_reduce(mn1[:], x2[:], axis=AXIS.X,
                                            op=ALU.min)

                # ---- per-l constants (all [128,1]) ----------------------
                dd = cpool.tile([128, 1], F32, tag="dd")
                nc.vector.tensor_tensor(dd[:], mx1[:], mn1[:], op=ALU.subtract)
                s1 = cpool.tile([128, 1], F32, tag="s1")
                # s1 = max(d,1e-6) * (1/15) -- HW tensor_scalar has no divide;
                # differs from the reference's d/15 by <=1 ulp (rare boundary flips)
                nc.vector.tensor_scalar(s1[:], dd[:], 1e-6, C15,
                                        op0=ALU.max, op1=ALU.mult)
                inv1 = cpool.tile([128, 1], F32, tag="inv1")
                nc.vector.reciprocal(inv1[:], s1[:])
                nb1 = cpool.tile([128, 1], F32, tag="nb1")
                # nb1 = -(mn1 * inv1): bias for the fused ACT affine
                nc.vector.tensor_scalar(nb1[:], mn1[:], inv1[:, 0:1], -1.0,
                                        op0=ALU.mult, op1=ALU.mult)

                # ---- quantize + dequant + store ------------------------
                # per-half on ACT; the last groups run dequant quartered on
                # DVE (idle post-reduce) so the final drain chain is short.
                nsub = 4 if group == n_groups - 1 else 1
                sub = HALF // nsub
                qs, os_ = [], []
                for b in range(B):
                    # q = rne(x*inv1 + nb1) via f32->i8 conversion.  Both
                    # quantizes run before any dequant so x2 (the big f32
                    # tile) is freed as early as possible — x-buffer reuse
                    # gates the input DMA stream and thus the reduces.
                    q = qpool.tile([128, HALF], I8, tag="q")
                    o = opool.tile([128, HALF], F16, tag="o")
                    xh = x2[:, b * HALF:(b + 1) * HALF]
                    for s in range(nsub):
                        c0, c1 = s * sub, (s + 1) * sub
                        nc.scalar.activation(q[:, c0:c1], xh[:, c0:c1],
                                             ACTF.Identity,
                                             bias=nb1[:, 0:1],
                                             scale=inv1[:, 0:1])
                    qs.append(q)
                    os_.append(o)
                for b in range(B):
                    q, o = qs[b], os_[b]
                    o3 = o[:].rearrange("l (h d) -> l h d", h=H)
                    for s in range(nsub):
                        c0, c1 = s * sub, (s + 1) * sub
                        h0, h1 = s * (H // nsub), (s + 1) * (H // nsub)
                        # out = q*s1 + mn1, cast to fp16
                        if tail:
                            nc.vector.tensor_scalar(o[:, c0:c1], q[:, c0:c1],
                                                    s1[:, 0:1], mn1[:, 0:1],
                                                    op0=ALU.mult, op1=ALU.add)
                        else:
                            nc.scalar.activation(o[:, c0:c1], q[:, c0:c1],
                                                 ACTF.Identity,
                                                 bias=mn1[:, 0:1],
                                                 scale=s1[:, 0:1])
                        nc.scalar.dma_start(
                            out=out[ci, b, l0:l0 + LCHUNK, h0:h1, :],
                            in_=o3[:, h0:h1, :])
                group += 1

    nc.compile()
    return nc


def _get_nc(lc=LC):
    if lc not in _BUILD_CACHE:
        _BUILD_CACHE[lc] = _build(lc)
    return _BUILD_CACHE[lc]


def _make_in_maps(k_cache_f, v_cache_f):
    """Per-core inputs, l-major ([B, LC, H, D]) for contiguous DMA lines."""
    in_maps = []
    for c in range(N_CORES):
        sl = slice(c * LC, (c + 1) * LC)
        in_maps.append({
            "k": np.ascontiguousarray(
                k_cache_f[:, :, sl, :].transpose(0, 2, 1, 3)),
            "v": np.ascontiguousarray(
                v_cache_f[:, :, sl, :].transpose(0, 2, 1, 3)),
        })
    return in_maps


def _host_fix_rows(out, cache_idx, val, input_pos):
    """Exact (fp32, reference-op-order) outputs for the scattered rows."""
    f32 = np.float32
    val = np.asarray(val, dtype=np.float32)
    pos = [int(p) for p in np.asarray(input_pos)]
    # last write wins for duplicate positions
    posmap = {}
    for i, p in enumerate(pos):
        posmap[p] = i
    for p, i in posmap.items():
        row = val[:, :, i, :]                       # [B,H,D]
        mn = row.min()
        mx = row.max()
        s2 = f32(max(mx - mn, f32(1e-6)) / f32(15))
        z2 = f32(mn + f32(s2 * f32(8)))
        t = ((row - mn) / s2).astype(np.float32)
        q = np.clip(np.round(t), 0, 15).astype(np.float32)
        out[cache_idx, :, :, p, :] = ((q - f32(8)) * s2).astype(np.float32) + z2


def kernel(k_cache_f, v_cache_f, k_val, v_val, input_pos):
    k_cache_f = np.asarray(k_cache_f, dtype=np.float32)
    v_cache_f = np.asarray(v_cache_f, dtype=np.float32)
    nc = _get_nc()
    in_maps = _make_in_maps(k_cache_f, v_cache_f)
    res = run_bass_kernel_spmd(nc, in_maps, list(range(N_CORES)))
    out = np.empty((2, B, H, L, D), dtype=np.float32)
    for c in range(N_CORES):
        sl = slice(c * LC, (c + 1) * LC)
        # [2, B, LC, H, D] fp16 -> [2, B, H, LC, D] f32
        out[:, :, :, sl, :] = res.results[c]["out"].transpose(0, 1, 3, 2, 4)
    _host_fix_rows(out, 0, k_val, input_pos)
    _host_fix_rows(out, 1, v_val, input_pos)
    return out


# revision 13
# speedup vs baseline: 1.1653x; 1.1653x over previous
"""Trainium2 Bass kernel for nn_KVCacheHybrid (quantized KV-cache scatter-update).

Reference semantics (per cache, k and v independently):
  1. 4-bit affine quantize along L (scales/zeros reduce over B,H,D per l)
  2. dequantize, scatter new rows at input_pos, re-quantize, dequantize.

Key observations that shape this kernel:
  * After the first quantize/dequant round-trip, codes 0 and 15 are attained in
    every l-slice, so the second-pass min/max for non-updated l are exactly the
    dequant grid endpoints; the second-pass scale/zero differ from the first by
    <= ~2 ulp (a ~1e-5 absolute output shift).  At fp16 output precision that
    is invisible, so this kernel reuses (s1, mn1) directly: out = q*s1 + mn1.
  * For non-updated l the second-pass codes equal the first-pass codes, so the
    device only computes q = rne((x - mn1) / s1) and the affine above.
  * Rows at input_pos depend only on k_val/v_val (0.5 MB) — computed exactly on
    the host and spliced into the gathered output.

Sharding: L axis across 8 cores (512 l's each).  The per-l reduction is then
fully core-local — no collectives.

Performance structure (vs the 244us h-major/f32 variant):
  * Inputs are uploaded l-major ([B, LC, H, D]) so each DMA partition line is
    H*D*4 = 16 KiB contiguous; the h-major layout produced 512 B packets which
    saturated the 16 DMA engines on per-packet overhead (~30ns/packet).
  * Output is written as fp16, halving write traffic; values are exact
    q*s1+mn1 rounded to fp16 (rel err ~2^-11, far inside the 2e-2 gate).
    The host upcasts while re-transposing.
  * f32->int8 output conversion on ACT/DVE rounds to nearest-even (verified on
    HW, including ties), so quantize is ONE fused op: q_i8 = Act(x*inv1 + nb1);
    no separate magic-constant rounding pass.
  * Work is issued per 2 MB b-half ([128 l, 4096]): DVE runs min/max reduces
    (4.4us each, ~141us total — the compute wall), ACT runs quantize (f32->i8)
    and dequant (i8->f16).  Both quantizes of a group run before its dequants
    so the x tiles free early (x-buffer reuse gates the input DMA stream).
    The first reduce is split so DVE starts on the first 1 MB; the last
    group's dequant/store is quartered and run on DVE to shorten the drain.
"""

import numpy as np
from contextlib import ExitStack

import concourse.bass as bass
import concourse.bacc as bacc
import concourse.tile as tile
from concourse import mybir
from concourse.bass_utils import run_bass_kernel_spmd

F32 = mybir.dt.float32
F16 = mybir.dt.float16
I8 = mybir.dt.int8
ALU = mybir.AluOpType
AXIS = mybir.AxisListType
ACTF = mybir.ActivationFunctionType

B, H, L, D = 2, 32, 4096, 128
N_CORES = 8
LC = L // N_CORES          # 512 l's per core
LCHUNK = 128               # l's per partition-tile
HALF = H * D               # 4096 elements per half-line (one b)
C15 = float(np.float32(1.0 / 15.0))

_BUILD_CACHE = {}


def _build(lc=LC):
    """Builds the per-core SPMD program; identical on all cores."""
    nc = bacc.Bacc("TRN2", target_bir_lowering=False, debug=False,
                   num_devices=N_CORES)
    k = nc.dram_tensor("k", [B, lc, H, D], F32, kind="ExternalInput").ap()
    v = nc.dram_tensor("v", [B, lc, H, D], F32, kind="ExternalInput").ap()
    out = nc.dram_tensor("out", [2, B, lc, H, D], F16,
                         kind="ExternalOutput").ap()

    n_chunks = lc // LCHUNK
    n_groups = 2 * n_chunks

    with tile.TileContext(nc) as tc, ExitStack() as ctx:
        xpool = ctx.enter_context(tc.tile_pool(name="x", bufs=8))
        qpool = ctx.enter_context(tc.tile_pool(name="q", bufs=4))
        opool = ctx.enter_context(tc.tile_pool(name="o", bufs=4))
        cpool = ctx.enter_context(tc.tile_pool(name="c", bufs=3))

        group = 0
        for ci, src in enumerate((k, v)):
            for lchunk in range(n_chunks):
                l0 = lchunk * LCHUNK
                # tail groups: DVE is idle once reduces drain; move dequant
                # there to keep ACT off the critical path at the end.
                tail = group >= n_groups - 2

                # ---- load per-b halves (16 KiB contiguous lines) --------
                xs = []
                pm = cpool.tile([128, 2 * B], F32, tag="pm")
                for b in range(B):
                    x2 = xpool.tile([128, HALF], F32, tag="x")
                    x3 = x2[:].rearrange("l (h d) -> l h d", h=H)
                    if group == 0 and b == 0:
                        # split the first load+reduce so DVE starts on the
                        # first 1 MB instead of waiting for the full 2 MB.
                        pm0 = cpool.tile([128, 4], F32, tag="pm0")
                        for s in range(2):
                            h0, h1 = s * (H // 2), (s + 1) * (H // 2)
                            c0, c1 = s * (HALF // 2), (s + 1) * (HALF // 2)
                            nc.sync.dma_start(
                                out=x3[:, h0:h1, :],
                                in_=src[b, l0:l0 + LCHUNK, h0:h1, :])
                            nc.vector.tensor_reduce(pm0[:, 2 * s:2 * s + 1],
                                                    x2[:, c0:c1],
                                                    axis=AXIS.X, op=ALU.max)
                            nc.vector.tensor_reduce(pm0[:, 2 * s + 1:2 * s + 2],
                                                    x2[:, c0:c1],
                                                    axis=AXIS.X, op=ALU.min)
                        nc.vector.tensor_tensor(pm[:, 0:1], pm0[:, 0:1],
                                                pm0[:, 2:3], op=ALU.max)
                        nc.vector.tensor_tensor(pm[:, 1:2], pm0[:, 1:2],
                                                pm0[:, 3:4], op=ALU.min)
                    else:
                        nc.sync.dma_start(out=x3,
                                          in_=src[b, l0:l0 + LCHUNK, :, :])
                        # per-half partial min/max -> pm columns
                        nc.vector.tensor_reduce(pm[:, 2 * b:2 * b + 1], x2[:],
                                                axis=AXIS.X, op=ALU.max)
                        nc.vector.tensor_reduce(pm[:, 2 * b + 1:2 * b + 2],
                                                x2[:],
                                                axis=AXIS.X, op=ALU.min)
                    xs.append(x2)

                # ---- per-l constants (all [128,1]) ----------------------
                mx1 = cpool.tile([128, 1], F32, tag="mx1")
                mn1 = cpool.tile([128, 1], F32, tag="mn1")
                nc.vector.tensor_tensor(mx1[:], pm[:, 0:1], pm[:, 2:3],
                                        op=ALU.max)
                nc.vector.tensor_tensor(mn1[:], pm[:, 1:2], pm[:, 3:4],
                                        op=ALU.min)
                dd = cpool.tile([128, 1], F32, tag="dd")
                nc.vector.tensor_tensor(dd[:], mx1[:], mn1[:], op=ALU.subtract)
                s1 = cpool.tile([128, 1], F32, tag="s1")
                # s1 = max(d,1e-6) * (1/15) -- HW tensor_scalar has no divide;
                # differs from the reference's d/15 by <=1 ulp (rare boundary flips)
                nc.vector.tensor_scalar(s1[:], dd[:], 1e-6, C15,
                                        op0=ALU.max, op1=ALU.mult)
                inv1 = cpool.tile([128, 1], F32, tag="inv1")
                nc.vector.reciprocal(inv1[:], s1[:])
                nb1 = cpool.tile([128, 1], F32, tag="nb1")
                # nb1 = -(mn1 * inv1): bias for the fused ACT affine
                nc.vector.tensor_scalar(nb1[:], mn1[:], inv1[:, 0:1], -1.0,
                                        op0=ALU.mult, op1=ALU.mult)

                # ---- quantize + dequant + store ------------------------
                nsub = 4 if group == n_groups - 1 else 1
                sub = HALF // nsub
                qs, os_ = [], []
                for b in range(B):
                    # q = rne(x*inv1 + nb1) via f32->i8 conversion.  Both
                    # quantizes run before any dequant so the x tiles free
                    # early — x-buffer reuse gates the input DMA stream.
                    q = qpool.tile([128, HALF], I8, tag="q")
                    o = opool.tile([128, HALF], F16, tag="o")
                    for s in range(nsub):
                        c0, c1 = s * sub, (s + 1) * sub
                        nc.scalar.activation(q[:, c0:c1], xs[b][:, c0:c1],
                                             ACTF.Identity,
                                             bias=nb1[:, 0:1],
                                             scale=inv1[:, 0:1])
                    qs.append(q)
                    os_.append(o)
                for b in range(B):
                    q, o = qs[b], os_[b]
                    o3 = o[:].rearrange("l (h d) -> l h d", h=H)
                    for s in range(nsub):
                        c0, c1 = s * sub, (s + 1) * sub
                        h0, h1 = s * (H // nsub), (s + 1) * (H // nsub)
                        # out = q*s1 + mn1, cast to fp16
                        if tail:
                            nc.vector.tensor_scalar(o[:, c0:c1], q[:, c0:c1],
                                                    s1[:, 0:1], mn1[:, 0:1],
                                                    op0=ALU.mult, op1=ALU.add)
                        else:
                            nc.scalar.activation(o[:, c0:c1], q[:, c0:c1],
                                                 ACTF.Identity,
                                                 bias=mn1[:, 0:1],
                                                 scale=s1[:, 0:1])
                        nc.scalar.dma_start(
                            out=out[ci, b, l0:l0 + LCHUNK, h0:h1, :],
                            in_=o3[:, h0:h1, :])
                group += 1

    nc.compile()
    return nc


def _get_nc(lc=LC):
    if lc not in _BUILD_CACHE:
        _BUILD_CACHE[lc] = _build(lc)
    return _BUILD_CACHE[lc]


def _make_in_maps(k_cache_f, v_cache_f):
    """Per-core inputs, l-major ([B, LC, H, D]) for contiguous DMA lines."""
    in_maps = []
    for c in range(N_CORES):
        sl = slice(c * LC, (c + 1) * LC)
        in_maps.append({
            "k": np.ascontiguousarray(
                k_cache_f[:, :, sl, :].transpose(0, 2, 1, 3)),
            "v": np.ascontiguousarray(
                v_cache_f[:, :, sl, :].transpose(0, 2, 1, 3)),
        })
    return in_maps


def _host_fix_rows(out, cache_idx, val, input_pos):
    """Exact (fp32, reference-op-order) outputs for the scattered rows."""
    f32 = np.float32
    val = np.asarray(val, dtype=np.float32)
    pos = [int(p) for p in np.asarray(input_pos)]
    # last write wins for duplicate positions
    posmap = {}
    for i, p in enumerate(pos):
        posmap[p] = i
    for p, i in posmap.items():
        row = val[:, :, i, :]                       # [B,H,D]
        mn = row.min()
        mx = row.max()
        s2 = f32(max(mx - mn, f32(1e-6)) / f32(15))
        z2 = f32(mn + f32(s2 * f32(8)))
        t = ((row - mn) / s2).astype(np.float32)
        q = np.clip(np.round(t), 0, 15).astype(np.float32)
        out[cache_idx, :, :, p, :] = ((q - f32(8)) * s2).astype(np.float32) + z2


def kernel(k_cache_f, v_cache_f, k_val, v_val, input_pos):
    k_cache_f = np.asarray(k_cache_f, dtype=np.float32)
    v_cache_f = np.asarray(v_cache_f, dtype=np.float32)
    nc = _get_nc()
    in_maps = _make_in_maps(k_cache_f, v_cache_f)
    res = run_bass_kernel_spmd(nc, in_maps, list(range(N_CORES)))
    out = np.empty((2, B, H, L, D), dtype=np.float32)
    for c in range(N_CORES):
        sl = slice(c * LC, (c + 1) * LC)
        # [2, B, LC, H, D] fp16 -> [2, B, H, LC, D] f32
        out[:, :, :, sl, :] = res.results[c]["out"].transpose(0, 1, 3, 2, 4)
    _host_fix_rows(out, 0, k_val, input_pos)
    _host_fix_rows(out, 1, v_val, input_pos)
    return out
